# revision 28
# baseline (speedup 1.0000x reference)
"""DeepWalk hierarchical-softmax scoring kernel for 8 Trainium2 NeuronCores.

Computation (mirrors the nn.Module reference):
    path = heap ancestors of leaf u_k           (L ~ 19-20 static ints)
    emd  = emd_weight[v_j]                      [128]
    hv   = hs_weight[path]                      [L, 128]
    out  = -prod(log_sigmoid(hv @ emd))         scalar f32

Distribution: full replication (the batch-size-1 degenerate case of the
hint's "batch many walks per device for data parallelism"). Both tables
are staged whole into every core's HBM; each core runs the complete
lookup + score locally and core 0's scalar is returned. For a single
walk this strictly dominates model-parallel sharding: any partitioning
of the tables forces at least one cross-core combine, and a collective
costs ~15us flat on this part — an order of magnitude more than the
entire computation. With replication the kernel is three DMAs, one
indirect gather, one fused multiply-reduce, four ACT ops and one PE
reduction, with zero communication.

Per-core dataflow:
  idx DMA -> idxt[L,1] (path row indices, one per partition)
  indirect DMA: hv[L,128] <- hs[path]  (single instruction, L descriptors)
  broadcast DMA: ev[L,128] <- emd[v_j] replicated (step-0 source AP)
  DVE scalar_tensor_tensor: pd[L,1] = sum_d hv*ev   (full dots)
  ACT: ea=Exp(-pd); sp=Ln(ea+1)=softplus(-dots); lt=Ln(sp)
  PE:  ps[1,1] = lt.T @ ones  (sum over the L partitions)
  ACT: res=Exp(ps) = prod(softplus) = (-1)^L * prod(logsig)
  out DMA <- res
(no softplus in this build's ACT tables; Exp/Ln share one table set,
prefetched by a dummy activation so the load hides under the gathers)
"""

import contextlib

import numpy as np

import concourse.bass as bass
import concourse.mybir as mybir
from concourse.bass_utils import run_bass_kernel_spmd

NUM_V = 1_000_000
EMD_DIM = 128
N_CORES = 8
F32 = mybir.dt.float32
I32 = mybir.dt.int32


def hs_path(u_k: int, num_V: int = NUM_V) -> list[int]:
    """Heap indices of all ancestors of leaf u_k, down-to-root (incl. 0)."""
    n = num_V - 1 + u_k
    path = []
    while n > 0:
        n = (n - 1) // 2
        path.append(n)
    return path


def build_module(v_j: int, u_k: int):
    """Build the per-core Bass module. v_j/u_k are compile-time constants,
    mirroring the reference where the path is a static int array."""
    path = hs_path(u_k)
    L = len(path)
    nc = bass.Bass(num_devices=N_CORES)

    emd = nc.dram_tensor("emd", [NUM_V, EMD_DIM], F32, kind="ExternalInput")
    hs = nc.dram_tensor("hs", [NUM_V - 1, EMD_DIM], F32, kind="ExternalInput")
    idx = nc.dram_tensor("idx", [1, L], I32, kind="ExternalInput")
    out = nc.dram_tensor("out", [1, 1], F32, kind="ExternalOutput")

    ctx = contextlib.ExitStack()
    with ctx:
        idxt = ctx.enter_context(nc.sbuf_tensor("idxt", [L, 1], I32))
        hv = ctx.enter_context(nc.sbuf_tensor("hv", [L, EMD_DIM], F32))
        ev = ctx.enter_context(nc.sbuf_tensor("ev", [L, EMD_DIM], F32))
        tmp = ctx.enter_context(nc.sbuf_tensor("tmp", [L, EMD_DIM], F32))
        pd = ctx.enter_context(nc.sbuf_tensor("pd", [L, 1], F32))
        ea = ctx.enter_context(nc.sbuf_tensor("ea", [L, 1], F32))
        sp = ctx.enter_context(nc.sbuf_tensor("sp", [L, 1], F32))
        lt = ctx.enter_context(nc.sbuf_tensor("lt", [L, 1], F32))
        res = ctx.enter_context(nc.sbuf_tensor("res", [1, 1], F32))
        warm = ctx.enter_context(nc.sbuf_tensor("warm", [1, 1], F32))
        ps = ctx.enter_context(nc.psum_tensor("ps", [1, 1], F32))
        pw = ctx.enter_context(nc.psum_tensor("pw", [1, 1], F32))
        dma_sem = ctx.enter_context(nc.semaphore("dma_sem"))
        idx_sem = ctx.enter_context(nc.semaphore("idx_sem"))
        g_sem = ctx.enter_context(nc.semaphore("g_sem"))
        v_sem = ctx.enter_context(nc.semaphore("v_sem"))
        s_sem = ctx.enter_context(nc.semaphore("s_sem"))
        t_sem = ctx.enter_context(nc.semaphore("t_sem"))
        block = ctx.enter_context(nc.Block())

        @block.sync
        def _(sync):
            # path row indices -> one per partition (for the indirect gather)
            sync.dma_start(out=idxt[:, :], in_=idx[0:1, :]).then_inc(idx_sem, 16)
            # center embedding row, replicated across the L partitions
            sync.dma_start(
                out=ev[:, :], in_=emd[v_j : v_j + 1, :].broadcast_to([L, EMD_DIM])
            ).then_inc(dma_sem, 16)

            # final scalar out
            sync.wait_ge(s_sem, 5)
            sync.dma_start(out=out[:, :], in_=res[:, :]).then_inc(dma_sem, 16)

        @block.gpsimd
        def _(gpsimd):
            # gather all L path rows in ONE indirect DMA: partition l reads
            # row idxt[l] (walrus requires the index table in SBUF)
            gpsimd.wait_ge(idx_sem, 16)
            gpsimd.indirect_dma_start(
                out=hv[:, :],
                out_offset=None,
                in_=hs[:, :],
                in_offset=bass.IndirectOffsetOnAxis(ap=idxt[:, :1], axis=0),
            ).then_inc(g_sem, 16)

        @block.vector
        def _(vector):
            # pd[l] = sum_d hv[l,d] * ev[l,d]  — the complete dot products
            vector.wait_ge(dma_sem, 16)
            vector.wait_ge(g_sem, 16)
            vector.scalar_tensor_tensor(
                out=tmp[:, :],
                in0=hv[:, :],
                scalar=1.0,
                in1=ev[:, :],
                op0=mybir.AluOpType.mult,
                op1=mybir.AluOpType.mult,
                accum_out=pd[:, :],
            ).then_inc(v_sem, 1)

        @block.scalar
        def _(scalar):
            # Dummy activation issued before any wait: triggers the ACT
            # table-set load (~2.7us) concurrently with the gather phase.
            scalar.activation(
                warm[:, :],
                nc.const_aps.tensor(0.0, (1, 1)),
                mybir.ActivationFunctionType.Exp,
            ).then_inc(s_sem, 1)

            # sp = softplus(-dots) = log(exp(-dots) + 1) = -log_sigmoid(dots)
            scalar.wait_ge(v_sem, 1)
            scalar.activation(
                ea[:, :],
                pd[:, :],
                mybir.ActivationFunctionType.Exp,
                scale=-1.0,
            ).then_inc(s_sem, 1)
            # ACT pipeline does not forward: same-engine RAW needs waits
            scalar.wait_ge(s_sem, 2)
            scalar.activation(
                sp[:, :],
                ea[:, :],
                mybir.ActivationFunctionType.Ln,
                bias=1.0,
            ).then_inc(s_sem, 1)
            scalar.wait_ge(s_sem, 3)
            scalar.activation(
                lt[:, :],
                sp[:, :],
                mybir.ActivationFunctionType.Ln,
            ).then_inc(s_sem, 1)

            # res = exp(sum_l ln(sp_l)) = prod(sp) = (-1)^(L+1) * answer
            scalar.wait_ge(t_sem, 1)
            scalar.activation(
                res[:, :],
                ps[:, :],
                mybir.ActivationFunctionType.Exp,
            ).then_inc(s_sem, 1)

        @block.tensor
        def _(tensor):
            # dummy matmul before any wait: raises the PE p-state and loads
            # the stationary `ones` column during the gather phase
            nc.tensor.matmul(
                out=pw[:, :],
                lhsT=nc.const_aps.tensor(1.0, (L, 1)),
                rhs=nc.const_aps.tensor(1.0, (L, 1)),
                start=True,
                stop=True,
            )
            # sum over the L partitions: ps = ones.T @ lt
            tensor.wait_ge(s_sem, 4)
            nc.tensor.matmul(
                out=ps[:, :],
                lhsT=nc.const_aps.tensor(1.0, (L, 1)),
                rhs=lt[:, :],
                start=True,
                stop=True,
            ).then_inc(t_sem, 1)

    # res = prod(sp) = (-1)^L prod(logsig); answer = -prod(logsig), so for odd
    # L the answer is res itself, for even L it is -res (host applies sign).
    sign = 1.0 if L % 2 == 1 else -1.0
    return nc, L, sign


_cache: dict = {}


def _get_module(v_j: int, u_k: int):
    key = (v_j, u_k)
    if key not in _cache:
        _cache[key] = build_module(v_j, u_k)
    return _cache[key]


def shard_inputs(emd_np: np.ndarray, hs_np: np.ndarray, u_k: int):
    idx_row = np.asarray(hs_path(u_k), dtype=np.int32).reshape(1, -1)
    emd_c = np.ascontiguousarray(emd_np)
    hs_c = np.ascontiguousarray(hs_np)
    return [{"emd": emd_c, "hs": hs_c, "idx": idx_row} for _ in range(N_CORES)]


def kernel(v_j, u_k, emd_weight, hs_weight) -> np.ndarray:
    v_j = int(v_j)
    u_k = int(u_k)
    emd_np = np.asarray(emd_weight, dtype=np.float32)
    hs_np = np.asarray(hs_weight, dtype=np.float32)
    assert emd_np.shape == (NUM_V, EMD_DIM), emd_np.shape
    assert hs_np.shape == (NUM_V - 1, EMD_DIM), hs_np.shape

    nc, L, sign = _get_module(v_j, u_k)
    in_maps = shard_inputs(emd_np, hs_np, u_k)
    results = run_bass_kernel_spmd(nc, in_maps, list(range(N_CORES))).results
    val = sign * float(results[0]["out"][0, 0])
    return np.float32(val)


# revision 29
# speedup vs baseline: 1.0919x; 1.0919x over previous
"""DeepWalk hierarchical-softmax scoring kernel for 8 Trainium2 NeuronCores.

Computation (mirrors the nn.Module reference):
    path = heap ancestors of leaf u_k           (L ~ 19-20 static ints)
    emd  = emd_weight[v_j]                      [128]
    hv   = hs_weight[path]                      [L, 128]
    out  = -prod(log_sigmoid(hv @ emd))         scalar f32

Distribution: full replication (the batch-size-1 degenerate case of the
hint's "batch many walks per device for data parallelism"). Both tables
are staged whole into every core's HBM; each core runs the complete
lookup + score locally and core 0's scalar is returned. For a single
walk this strictly dominates model-parallel sharding: any partitioning
of the tables forces at least one cross-core combine, and a collective
costs ~15us flat on this part — an order of magnitude more than the
entire computation. With replication the kernel is three DMAs, one
indirect gather, one fused multiply-reduce, four ACT ops and one PE
reduction, with zero communication.

Per-core dataflow:
  idx DMA -> idxt[L,1] (path row indices, one per partition)
  indirect DMA: hv[L,128] <- hs[path]  (single instruction, L descriptors)
  broadcast DMA: ev[L,128] <- emd[v_j] replicated (step-0 source AP)
  DVE scalar_tensor_tensor: pd[L,1] = sum_d hv*ev   (full dots)
  ACT: ea=Exp(-pd); sp=Ln(ea+1)=softplus(-dots); lt=Ln(sp)
  PE:  ps[1,1] = lt.T @ ones  (sum over the L partitions)
  ACT: res=Exp(ps) = prod(softplus) = (-1)^L * prod(logsig)
  out DMA <- res
(no softplus in this build's ACT tables; Exp/Ln share one table set,
prefetched by a dummy activation so the load hides under the gathers)
"""

import contextlib

import numpy as np

import concourse.bass as bass
import concourse.mybir as mybir
from concourse.bass_utils import run_bass_kernel_spmd

NUM_V = 1_000_000
EMD_DIM = 128
N_CORES = 8
F32 = mybir.dt.float32
I32 = mybir.dt.int32


def hs_path(u_k: int, num_V: int = NUM_V) -> list[int]:
    """Heap indices of all ancestors of leaf u_k, down-to-root (incl. 0)."""
    n = num_V - 1 + u_k
    path = []
    while n > 0:
        n = (n - 1) // 2
        path.append(n)
    return path


def build_module(v_j: int, u_k: int):
    """Build the per-core Bass module. v_j/u_k are compile-time constants,
    mirroring the reference where the path is a static int array."""
    path = hs_path(u_k)
    L = len(path)
    nc = bass.Bass(num_devices=N_CORES)

    emd = nc.dram_tensor("emd", [NUM_V, EMD_DIM], F32, kind="ExternalInput")
    hs = nc.dram_tensor("hs", [NUM_V - 1, EMD_DIM], F32, kind="ExternalInput")
    idx = nc.dram_tensor("idx", [1, L], I32, kind="ExternalInput")
    out = nc.dram_tensor("out", [1, 1], F32, kind="ExternalOutput")

    ctx = contextlib.ExitStack()
    with ctx:
        idxt = ctx.enter_context(nc.sbuf_tensor("idxt", [L, 1], I32))
        hv = ctx.enter_context(nc.sbuf_tensor("hv", [L, EMD_DIM], F32))
        ev = ctx.enter_context(nc.sbuf_tensor("ev", [L, EMD_DIM], F32))
        tmp = ctx.enter_context(nc.sbuf_tensor("tmp", [L, EMD_DIM], F32))
        pd = ctx.enter_context(nc.sbuf_tensor("pd", [L, 1], F32))
        ea = ctx.enter_context(nc.sbuf_tensor("ea", [L, 1], F32))
        sp = ctx.enter_context(nc.sbuf_tensor("sp", [L, 1], F32))
        lt = ctx.enter_context(nc.sbuf_tensor("lt", [L, 1], F32))
        res = ctx.enter_context(nc.sbuf_tensor("res", [1, 1], F32))
        warm = ctx.enter_context(nc.sbuf_tensor("warm", [1, 1], F32))
        ps = ctx.enter_context(nc.psum_tensor("ps", [1, 1], F32))
        dma_sem = ctx.enter_context(nc.semaphore("dma_sem"))
        idx_sem = ctx.enter_context(nc.semaphore("idx_sem"))
        g_sem = ctx.enter_context(nc.semaphore("g_sem"))
        v_sem = ctx.enter_context(nc.semaphore("v_sem"))
        s_sem = ctx.enter_context(nc.semaphore("s_sem"))
        t_sem = ctx.enter_context(nc.semaphore("t_sem"))
        block = ctx.enter_context(nc.Block())

        @block.sync
        def _(sync):
            # center embedding row, replicated across the L partitions
            sync.dma_start(
                out=ev[:, :], in_=emd[v_j : v_j + 1, :].broadcast_to([L, EMD_DIM])
            ).then_inc(dma_sem, 16)

            # final scalar out
            sync.wait_ge(s_sem, 5)
            sync.dma_start(out=out[:, :], in_=res[:, :]).then_inc(dma_sem, 16)

        @block.gpsimd
        def _(gpsimd):
            # path row indices -> one per partition, via the Pool engine's own
            # SWDGE ring (cheaper fixed cost than HWDGE, and no cross-engine
            # hop into the indirect gather below)
            gpsimd.dma_start(out=idxt[:, :], in_=idx[0:1, :]).then_inc(idx_sem, 16)
            # gather all L path rows in ONE indirect DMA: partition l reads
            # row idxt[l] (walrus requires the index table in SBUF)
            gpsimd.wait_ge(idx_sem, 16)
            gpsimd.indirect_dma_start(
                out=hv[:, :],
                out_offset=None,
                in_=hs[:, :],
                in_offset=bass.IndirectOffsetOnAxis(ap=idxt[:, :1], axis=0),
            ).then_inc(g_sem, 16)

        @block.vector
        def _(vector):
            # pd[l] = sum_d hv[l,d] * ev[l,d]  — the complete dot products
            vector.wait_ge(dma_sem, 16)
            vector.wait_ge(g_sem, 16)
            vector.scalar_tensor_tensor(
                out=tmp[:, :],
                in0=hv[:, :],
                scalar=1.0,
                in1=ev[:, :],
                op0=mybir.AluOpType.mult,
                op1=mybir.AluOpType.mult,
                accum_out=pd[:, :],
            ).then_inc(v_sem, 1)

        @block.scalar
        def _(scalar):
            # Dummy activation issued before any wait: triggers the ACT
            # table-set load (~2.7us) concurrently with the gather phase.
            scalar.activation(
                warm[:, :],
                nc.const_aps.tensor(0.0, (1, 1)),
                mybir.ActivationFunctionType.Exp,
            ).then_inc(s_sem, 1)

            # sp = softplus(-dots) = log(exp(-dots) + 1) = -log_sigmoid(dots)
            scalar.wait_ge(v_sem, 1)
            scalar.activation(
                ea[:, :],
                pd[:, :],
                mybir.ActivationFunctionType.Exp,
                scale=-1.0,
            ).then_inc(s_sem, 1)
            # ACT pipeline does not forward: same-engine RAW needs waits
            scalar.wait_ge(s_sem, 2)
            scalar.activation(
                sp[:, :],
                ea[:, :],
                mybir.ActivationFunctionType.Ln,
                bias=1.0,
            ).then_inc(s_sem, 1)
            scalar.wait_ge(s_sem, 3)
            scalar.activation(
                lt[:, :],
                sp[:, :],
                mybir.ActivationFunctionType.Ln,
            ).then_inc(s_sem, 1)

            # res = exp(sum_l ln(sp_l)) = prod(sp) = (-1)^(L+1) * answer
            scalar.wait_ge(t_sem, 1)
            scalar.activation(
                res[:, :],
                ps[:, :],
                mybir.ActivationFunctionType.Exp,
            ).then_inc(s_sem, 1)

        @block.tensor
        def _(tensor):
            # sum over the L partitions: ps = lt.T @ ones
            tensor.wait_ge(s_sem, 4)
            nc.tensor.matmul(
                out=ps[:, :],
                lhsT=lt[:, :],
                rhs=nc.const_aps.tensor(1.0, (L, 1)),
                start=True,
                stop=True,
            ).then_inc(t_sem, 1)

    # res = prod(sp) = (-1)^L prod(logsig); answer = -prod(logsig), so for odd
    # L the answer is res itself, for even L it is -res (host applies sign).
    sign = 1.0 if L % 2 == 1 else -1.0
    return nc, L, sign


_cache: dict = {}


def _get_module(v_j: int, u_k: int):
    key = (v_j, u_k)
    if key not in _cache:
        _cache[key] = build_module(v_j, u_k)
    return _cache[key]


def shard_inputs(emd_np: np.ndarray, hs_np: np.ndarray, u_k: int):
    idx_row = np.asarray(hs_path(u_k), dtype=np.int32).reshape(1, -1)
    emd_c = np.ascontiguousarray(emd_np)
    hs_c = np.ascontiguousarray(hs_np)
    return [{"emd": emd_c, "hs": hs_c, "idx": idx_row} for _ in range(N_CORES)]


def kernel(v_j, u_k, emd_weight, hs_weight) -> np.ndarray:
    v_j = int(v_j)
    u_k = int(u_k)
    emd_np = np.asarray(emd_weight, dtype=np.float32)
    hs_np = np.asarray(hs_weight, dtype=np.float32)
    assert emd_np.shape == (NUM_V, EMD_DIM), emd_np.shape
    assert hs_np.shape == (NUM_V - 1, EMD_DIM), hs_np.shape

    nc, L, sign = _get_module(v_j, u_k)
    in_maps = shard_inputs(emd_np, hs_np, u_k)
    results = run_bass_kernel_spmd(nc, in_maps, list(range(N_CORES))).results
    val = sign * float(results[0]["out"][0, 0])
    return np.float32(val)
